# revision 7
# baseline (speedup 1.0000x reference)
"""Trainium2 Bass kernel for ContrastiveNet loss.

Algorithm (per core k of 8, SPMD):
  - host: xt_rot = x.T rolled so core k's 512 anchor rows sit at columns 0..511
  - device: cast xT->bf16, norms via squares + ones-matmul colsum,
    gram G = Xblk @ X.T in bf16 (PE), sim = G * invw_row * invw_col (DVE, ->bf16),
    per-pair logit gather via gpsimd.local_scatter (per-partition scatter of each
    sim row into pair-slot layout, duplicate columns handled by chained levels),
    masked exp/logsumexp (ACT+DVE), per-core partial sum -> [1,1].
  - host: sum 8 partials / P.
"""
import os
import sys
import numpy as np
import ml_dtypes

try:
    import concourse  # noqa: F401
except ImportError:
    sys.path.insert(0, "/opt/trn_rl_repo")

from contextlib import ExitStack

import concourse.bass as bass
import concourse.tile as tile
from concourse import bacc, mybir
from concourse._compat import with_exitstack
from concourse.bass_utils import run_bass_kernel_spmd

BF16 = ml_dtypes.bfloat16
F32 = mybir.dt.float32
DBF = mybir.dt.bfloat16
I16 = mybir.dt.int16

B, D, J = 4096, 2048, 11
NCORES, RPC, NT, NKT = 8, 512, 4, 16  # rows/core, row-tiles/core, k-tiles
TEMP = 0.1
AF = mybir.ActivationFunctionType
ALU = mybir.AluOpType
AX = mybir.AxisListType


# ---------------------------------------------------------------- host prep
def build_plan(anchor_idx, pos_idx, neg_idx):
    """Scatter planes; plane0 column indices are per-core ROTATED by -512k."""
    r = anchor_idx.astype(np.int64)
    cols = np.concatenate([pos_idx[:, None], neg_idx], axis=1).astype(np.int64)
    P = r.shape[0]

    order = np.argsort(r, kind="stable")
    r_sorted = r[order]
    first = np.r_[True, r_sorted[1:] != r_sorted[:-1]]
    gid = np.cumsum(first) - 1
    rank_sorted = np.arange(P) - np.flatnonzero(first)[gid]
    srank = np.empty(P, np.int64)
    srank[order] = rank_sorted
    n_per_row = np.bincount(r, minlength=B)
    SP = int(max(n_per_row.max(), 1))
    NE = SP * J + (SP * J) % 2
    assert NE * 32 < 2**16

    er = np.repeat(r, J)
    ec = cols.ravel()
    eslot = np.repeat(srank, J) * J + np.tile(np.arange(J), P)
    key = er * B + ec
    o2 = np.argsort(key, kind="stable")
    k_sorted = key[o2]
    first2 = np.r_[True, k_sorted[1:] != k_sorted[:-1]]
    gid2 = np.cumsum(first2) - 1
    occ_sorted = np.arange(P * J) - np.flatnonzero(first2)[gid2]
    occ = np.empty(P * J, np.int64)
    occ[o2] = occ_sorted
    L = int(occ.max())

    eslot_sorted = eslot[o2]
    prev_slot_sorted = np.empty(P * J, np.int64)
    prev_slot_sorted[0] = -1
    prev_slot_sorted[1:] = eslot_sorted[:-1]
    prev_slot = np.empty(P * J, np.int64)
    prev_slot[o2] = prev_slot_sorted

    core = er // RPC
    t = (er % RPC) // 128
    pp = er % 128
    ec_rot = (ec - core * RPC) % B  # per-core rotated column index

    plane0 = np.full((NCORES, NT, 128, B), -1, np.int16)
    m0 = occ == 0
    plane0[core[m0], t[m0], pp[m0], ec_rot[m0]] = eslot[m0].astype(np.int16)

    planes = []
    for q in range(1, L + 1):
        pl = np.full((NCORES, NT, 128, NE), -1, np.int16)
        mq = occ == q
        pl[core[mq], t[mq], pp[mq], prev_slot[mq]] = eslot[mq].astype(np.int16)
        planes.append(pl)

    nmat = n_per_row.reshape(NCORES, NT, 128)
    maskplane = ((np.arange(NE)[None, None, None, :] // J) < nmat[..., None]).astype(BF16)
    pairmask = (np.arange(SP)[None, None, None, :] < nmat[..., None]).astype(BF16)
    return dict(plane0=plane0, planes=planes, maskplane=maskplane,
                pairmask=pairmask, SP=SP, NE=NE, L=L)


# ------------------------------------------------------------- device kernel
@with_exitstack
def _build(ctx: ExitStack, tc: "tile.TileContext", io: dict, SP: int, NE: int, L: int):
    nc = tc.nc
    xt, pl0_d, mask_d, pm_d, out_d = io["xt"], io["plane0"], io["mask"], io["pm"], io["out"]
    plq_d = [io[f"plane{q}"] for q in range(1, L + 1)]

    consts = ctx.enter_context(tc.tile_pool(name="consts", bufs=1))
    ones_bf = consts.tile([128, 1], DBF, tag="ones_bf")
    nc.vector.memset(ones_bf[:], 1.0)
    ones_f32r = consts.tile([1, 128], F32, tag="ones_f32r")
    nc.vector.memset(ones_f32r[:], 1.0)
    ones_f32c = consts.tile([128, 1], F32, tag="ones_f32c")
    nc.vector.memset(ones_f32c[:], 1.0)
    neg30 = consts.tile([128, 1], F32, tag="neg30")
    nc.vector.memset(neg30[:], -30.0)

    ypool = ctx.enter_context(tc.tile_pool(name="y", bufs=1))
    y = [ypool.tile([128, B], DBF, tag=f"y{kt}", name=f"y{kt}") for kt in range(NKT)]

    npool = ctx.enter_context(tc.tile_pool(name="norms", bufs=1))
    invw = npool.tile([1, B], F32, tag="invw")
    invw_bc = npool.tile([128, B], DBF, tag="invw_bc")
    invwT = npool.tile([128, NT], F32, tag="invwT")

    # ---- phase 1: load, cast, squares, colsum
    with tc.tile_pool(name="p1psum", bufs=1, space="PSUM") as p1psum, \
         tc.tile_pool(name="stage", bufs=2) as stpool, \
         tc.tile_pool(name="sq", bufs=2) as sqpool:
        norm_ps = p1psum.tile([1, B], F32, tag="norm_ps")
        for kt in range(NKT):
            st = stpool.tile([128, B], F32, tag="stage")
            nc.sync.dma_start(st[:], xt[kt * 128:(kt + 1) * 128, :])
            sq = sqpool.tile([128, B], DBF, tag="sq")
            if kt % 2 == 0:
                nc.scalar.copy(y[kt][:], st[:])
                nc.vector.tensor_tensor(sq[:], st[:], st[:], ALU.mult)
            else:
                nc.vector.tensor_copy(y[kt][:], st[:])
                nc.scalar.activation(sq[:], st[:], AF.Square)
            for nch in range(8):
                nc.tensor.matmul(
                    norm_ps[:, nch * 512:(nch + 1) * 512],
                    lhsT=ones_bf[:, 0:1], rhs=sq[:, nch * 512:(nch + 1) * 512],
                    start=(kt == 0), stop=(kt == NKT - 1),
                )
        nc.scalar.copy(invw[:], norm_ps[:])

    # ---- phase 2: invw = sqrt(10/norm2) = sqrt(10)/||x|| (in-place on invw)
    nc.vector.reciprocal(invw[:], invw[:])
    nc.scalar.activation(invw[:], invw[:], AF.Sqrt, scale=1.0 / TEMP)
    with tc.tile_pool(name="p2psum", bufs=1, space="PSUM") as p2psum:
        psT = p2psum.tile([128, NT], F32, tag="psT")
        for mt in range(NT):
            nc.tensor.matmul(
                psT[:, mt:mt + 1],
                lhsT=invw[0:1, mt * 128:(mt + 1) * 128], rhs=ones_f32r[0:1, 0:1],
                start=True, stop=True,
            )
        nc.scalar.copy(invwT[:], psT[:])
        for nch in range(8):
            bc = p2psum.tile([128, 512], F32, tag="bc")
            nc.tensor.matmul(
                bc[:],
                lhsT=ones_f32r[0:1, :], rhs=invw[0:1, nch * 512:(nch + 1) * 512],
                start=True, stop=True,
            )
            nc.scalar.copy(invw_bc[:, nch * 512:(nch + 1) * 512], bc[:])

    # ---- phases 3+4: gram per (mt, half), fused normalize; scatter + loss per mt
    lpool = ctx.enter_context(tc.tile_pool(name="loss", bufs=1))
    acc4 = lpool.tile([128, NT], F32, tag="acc4")

    with tc.tile_pool(name="gpsum", bufs=2, space="PSUM") as gpsum, \
         tc.tile_pool(name="gbf", bufs=2) as gbfpool, \
         tc.tile_pool(name="pl", bufs=1) as plpool, \
         tc.tile_pool(name="slots", bufs=2) as slpool, \
         tc.tile_pool(name="elb", bufs=1) as elpool:
        for mt in range(NT):
            gbf = gbfpool.tile([128, B], DBF, tag="gbf")
            for half in range(2):
                gps = gpsum.tile([128, 2048], F32, tag="gram")
                for kt in range(NKT):
                    for nch in range(4):
                        nc.tensor.matmul(
                            gps[:, nch * 512:(nch + 1) * 512],
                            lhsT=y[kt][:, mt * 128:(mt + 1) * 128],
                            rhs=y[kt][:, half * 2048 + nch * 512: half * 2048 + (nch + 1) * 512],
                            start=(kt == 0), stop=(kt == NKT - 1),
                        )
                nc.vector.scalar_tensor_tensor(
                    gbf[:, half * 2048:(half + 1) * 2048], gps[:],
                    invwT[:, mt:mt + 1],
                    invw_bc[:, half * 2048:(half + 1) * 2048],
                    ALU.mult, ALU.mult,
                )

            # scatter chain
            pl0 = plpool.tile([128, B], I16, tag="pl0")
            nc.sync.dma_start(pl0[:], pl0_d[mt])
            s_lv = slpool.tile([128, NE], DBF, tag=f"slv0")
            nc.gpsimd.local_scatter(s_lv[:], gbf[:], pl0[:], 128, NE, B)
            s_all = slpool.tile([128, NE], DBF, tag="s_all")
            nc.vector.tensor_copy(s_all[:], s_lv[:])
            for q in range(1, L + 1):
                plq = plpool.tile([128, NE], I16, tag=f"plq{q}")
                nc.sync.dma_start(plq[:], plq_d[q - 1][mt])
                s_nx = slpool.tile([128, NE], DBF, tag=f"slv{q % 2 + 1}")
                nc.gpsimd.local_scatter(s_nx[:], s_lv[:], plq[:], 128, NE, NE)
                nc.vector.tensor_tensor(s_all[:], s_all[:], s_nx[:], ALU.add)
                s_lv = s_nx

            # masked exp / logsumexp / accumulate
            msk = elpool.tile([128, NE], DBF, tag="msk")
            nc.sync.dma_start(msk[:], mask_d[mt])
            pm = elpool.tile([128, SP], DBF, tag="pm")
            nc.sync.dma_start(pm[:], pm_d[mt])
            arg = elpool.tile([128, NE], F32, tag="arg")
            nc.vector.scalar_tensor_tensor(arg[:], s_all[:], 30.0, msk[:], ALU.add, ALU.mult)
            ebuf = elpool.tile([128, NE], F32, tag="ebuf")
            nc.scalar.activation(ebuf[:], arg[:], AF.Exp, bias=neg30[:, 0:1])
            denom = elpool.tile([128, SP], F32, tag="denom")
            e3 = ebuf[:, 0:SP * J].rearrange("p (s j) -> p s j", j=J)
            nc.vector.tensor_reduce(denom[:], e3, AX.X, ALU.add)
            lnd = elpool.tile([128, SP], F32, tag="lnd")
            nc.scalar.activation(lnd[:], denom[:], AF.Ln)
            diff = elpool.tile([128, SP], F32, tag="diff")
            l0 = s_all[:, 0:SP * J].rearrange("p (s j) -> p s j", j=J)[:, :, 0]
            nc.vector.scalar_tensor_tensor(diff[:], l0, -1.0, lnd[:], ALU.mult, ALU.add)
            scrap = elpool.tile([128, SP], F32, tag="scrap")
            nc.vector.scalar_tensor_tensor(
                scrap[:], diff[:], 1.0, pm[:], ALU.mult, ALU.mult,
                accum_out=acc4[:, mt:mt + 1],
            )

    # ---- phase 5: total
    with tc.tile_pool(name="p5psum", bufs=1, space="PSUM") as p5psum:
        tot = lpool.tile([128, 1], F32, tag="tot")
        nc.vector.tensor_reduce(tot[:], acc4[:], AX.X, ALU.add)
        ps = p5psum.tile([1, 1], F32, tag="ps_out")
        nc.tensor.matmul(ps[:], lhsT=tot[:], rhs=ones_f32c[:, 0:1],
                         start=True, stop=True)
        res = lpool.tile([1, 1], F32, tag="res")
        nc.scalar.copy(res[:], ps[:])
        nc.sync.dma_start(out_d[:], res[:])


def build_nc(SP, NE, L, enable_asserts=False):
    nc = bacc.Bacc("TRN2", target_bir_lowering=False, debug=False,
                   enable_asserts=enable_asserts, num_devices=NCORES)
    io = {
        "xt": nc.dram_tensor("xt", [D, B], F32, kind="ExternalInput").ap(),
        "plane0": nc.dram_tensor("plane0", [NT, 128, B], I16, kind="ExternalInput").ap(),
        "mask": nc.dram_tensor("mask", [NT, 128, NE], DBF, kind="ExternalInput").ap(),
        "pm": nc.dram_tensor("pm", [NT, 128, SP], DBF, kind="ExternalInput").ap(),
        "out": nc.dram_tensor("out", [1, 1], F32, kind="ExternalOutput").ap(),
    }
    for q in range(1, L + 1):
        io[f"plane{q}"] = nc.dram_tensor(
            f"plane{q}", [NT, 128, NE], I16, kind="ExternalInput").ap()
    with tile.TileContext(nc) as tc:
        _build(tc, io, SP, NE, L)
    nc.compile()
    return nc


def make_in_maps(x, plan):
    xT = np.ascontiguousarray(np.asarray(x, np.float32).T)
    in_maps = []
    for k in range(NCORES):
        m = {
            "xt": np.ascontiguousarray(np.roll(xT, -RPC * k, axis=1)),
            "plane0": plan["plane0"][k],
            "mask": plan["maskplane"][k],
            "pm": plan["pairmask"][k],
        }
        for q in range(1, plan["L"] + 1):
            m[f"plane{q}"] = plan["planes"][q - 1][k]
        in_maps.append(m)
    return in_maps


def kernel(**inputs):
    x = np.asarray(inputs["x"], np.float32)
    anchor_idx = np.asarray(inputs["anchor_idx"])
    pos_idx = np.asarray(inputs["pos_idx"])
    neg_idx = np.asarray(inputs["neg_idx"])
    P = anchor_idx.shape[0]

    plan = build_plan(anchor_idx, pos_idx, neg_idx)
    nc = build_nc(plan["SP"], plan["NE"], plan["L"])
    in_maps = make_in_maps(x, plan)
    res = run_bass_kernel_spmd(nc, in_maps, list(range(NCORES)))
    total = sum(float(res.results[k]["out"][0, 0]) for k in range(NCORES))
    return np.float32(total / P)
